# revision 10
# baseline (speedup 1.0000x reference)
"""Multi-head attention (B=4, L=2048, D=512, H=8) on 8 Trainium2 NeuronCores.

Sharding: data-parallel over batch (4) x tensor-parallel over heads (2 groups
of 4 heads). Core c handles batch c//2, head-group c%2. Each core computes
   Q^T/K^T (feature-major) and V (token-major) projections for its 4 heads,
   causal attention with scores kept transposed (S^T = K Q^T) so softmax'd
   probabilities feed the PV matmul directly as the moving operand,
   denominators via a ones-column appended to V, and a partial output
   projection over its 256 features.
Host sums the two head-group partials per batch and adds the output bias.

Compute is bf16 (fp32 accumulation in PSUM); softmax runs in fp32 on the
scalar engine with the 1/sqrt(hd) scale fused into the exp activation.
"""
import sys

sys.path.insert(0, "/opt/trn_rl_repo")

import numpy as np
import ml_dtypes

import concourse.bacc as bacc
import concourse.tile as tile
import concourse.mybir as mybir
from concourse.bass_utils import run_bass_kernel_spmd

BF16 = mybir.dt.bfloat16
F32 = mybir.dt.float32
Exp = mybir.ActivationFunctionType.Exp

B, L, D, H = 4, 2048, 512, 8
HD = D // H          # 64
DG = D // 2          # 256 features per head-group
NT = L // 128        # 16 token tiles
QG = 8               # q-tiles per pass
NPASS = NT // QG     # 2
EXACT_Z = True       # hi/lo split of 1/Z so normalization is fp32-exact

_CACHE = {}


def _chunks(t0, t1, step):
    """[t0, t1) split so no chunk crosses a multiple of `step`."""
    out = []
    t = t0
    while t < t1:
        nxt = min(t1, (t // step + 1) * step)
        out.append((t, nxt))
        t = nxt
    return out


def _build(causal):
    nc = bacc.Bacc("TRN2", target_bir_lowering=False, debug=False, num_devices=8)
    dt = nc.dram_tensor
    qb = dt("qb", [L, D], BF16, kind="ExternalInput").ap()
    kb = dt("kb", [L, D], BF16, kind="ExternalInput").ap()
    vb = dt("vb", [L, D], BF16, kind="ExternalInput").ap()
    wq = dt("wq", [D, DG], BF16, kind="ExternalInput").ap()
    wk = dt("wk", [D, DG], BF16, kind="ExternalInput").ap()
    wv = dt("wv", [D, DG], BF16, kind="ExternalInput").ap()
    bqk = dt("bqk", [128, 4], F32, kind="ExternalInput").ap()
    bv = dt("bv", [1, DG], BF16, kind="ExternalInput").ap()
    wo4 = dt("wo4", [HD, 4, D], BF16, kind="ExternalInput").ap()
    triu = dt("triu", [128, 128], BF16, kind="ExternalInput").ap()
    out = dt("out", [L, D], F32, kind="ExternalOutput").ap()

    with tile.TileContext(nc) as tc:
        with (
            tc.tile_pool(name="sb", bufs=1) as sb,
            tc.tile_pool(name="ps", space="PSUM", bufs=1) as ps,
        ):
            # ---- constants / weights -------------------------------------
            wq_sb = sb.tile([128, 4, DG], BF16, tag="wq")
            wk_sb = sb.tile([128, 4, DG], BF16, tag="wk")
            wv_sb = sb.tile([128, 4, DG], BF16, tag="wv")
            nc.sync.dma_start(wq_sb[:], wq.rearrange("(s p) n -> p s n", p=128))
            nc.sync.dma_start(wk_sb[:], wk.rearrange("(s p) n -> p s n", p=128))
            nc.sync.dma_start(wv_sb[:], wv.rearrange("(s p) n -> p s n", p=128))
            bqk_sb = sb.tile([128, 4], F32, tag="bqk")
            nc.sync.dma_start(bqk_sb[:], bqk[:])
            bv_sb = sb.tile([1, DG], BF16, tag="bv")
            nc.sync.dma_start(bv_sb[:], bv[:])
            wo_sb = sb.tile([HD, 4, D], BF16, tag="wo")
            nc.sync.dma_start(wo_sb[:], wo4[:])
            triu_sb = sb.tile([128, 128], BF16, tag="triu")
            nc.sync.dma_start(triu_sb[:], triu[:])
            ones_sb = sb.tile([128, 128], BF16, tag="ones")
            nc.vector.memset(ones_sb[:], 1.0)

            # ---- transposed activation loads (feature-major slabs) -------
            kT = []
            qT = []
            vT = []
            for s in range(4):
                t = sb.tile([128, L], BF16, tag="xT", bufs=12)
                nc.sync.dma_start(t[:], kb[:, s * 128 : (s + 1) * 128], transpose=True)
                kT.append(t)
            for s in range(4):
                t = sb.tile([128, L], BF16, tag="xT", bufs=12)
                nc.sync.dma_start(t[:], qb[:, s * 128 : (s + 1) * 128], transpose=True)
                qT.append(t)
            for s in range(4):
                t = sb.tile([128, L], BF16, tag="xT", bufs=12)
                nc.sync.dma_start(t[:], vb[:, s * 128 : (s + 1) * 128], transpose=True)
                vT.append(t)

            # ---- K^T / Q^T projections (feature-major outputs) -----------
            KT_sb = sb.tile([128, 2, L], BF16, tag="KT")
            QT_sb = sb.tile([128, 2, L], BF16, tag="QT")
            for xT, w_sb, bcol, dst in ((kT, wk_sb, 2, KT_sb), (qT, wq_sb, 0, QT_sb)):
                for m in range(2):
                    for n in range(4):
                        pqk = ps.tile([128, 512], F32, tag="ctx", bufs=2, name="pqk")
                        for s in range(4):
                            nc.tensor.matmul(
                                pqk[:],
                                w_sb[:, s, m * 128 : (m + 1) * 128],
                                xT[s][:, n * 512 : (n + 1) * 512],
                                start=(s == 0),
                                stop=(s == 3),
                            )
                        nc.vector.tensor_scalar_add(
                            dst[:, m, n * 512 : (n + 1) * 512],
                            pqk[:],
                            bqk_sb[:, bcol + m : bcol + m + 1],
                        )

            # ---- V projection (token-major, ones column for Z) -----------
            V_sb = sb.tile([128, 4, NT, HD + 1], BF16, tag="V")
            for tt in range(NT):
                pv = ps.tile([128, DG], F32, tag="ctx", bufs=2, name="pv")
                for s in range(4):
                    nc.tensor.matmul(
                        pv[:],
                        vT[s][:, tt * 128 : (tt + 1) * 128],
                        wv_sb[:, s, :],
                        start=(s == 0),
                        stop=False,
                    )
                nc.tensor.matmul(
                    pv[:], ones_sb[0:1, :], bv_sb[:], start=False, stop=True
                )
                nc.vector.tensor_copy(
                    V_sb[:, :, tt, 0:HD],
                    pv[:].rearrange("p (h d) -> p h d", h=4),
                )
            nc.vector.memset(V_sb[:, :, :, HD : HD + 1], 1.0)

            # ---- attention ----------------------------------------------
            for j in range(NPASS):
                ctxn = [
                    sb.tile([HD, QG * 128], BF16, tag=f"ctxn{h}", bufs=2, name=f"ctxn{h}")
                    for h in range(4)
                ]
                for hp in range(2):
                    kmax = QG * (j + 1) if causal else NT
                    ctxs = [
                        ps.tile([HD + 1, QG * 128], F32, tag="ctx", bufs=2, name=f"ctx{i}")
                        for i in range(2)
                    ]
                    for ki in range(kmax):
                        vq0 = max(ki - QG * j, 0) if causal else 0
                        st = ps.tile([128, 2048], F32, tag="s", bufs=1)
                        for hh in range(2):
                            lhs = KT_sb[
                                hh * 64 : (hh + 1) * 64,
                                hp,
                                ki * 128 : (ki + 1) * 128,
                            ]
                            for c0, c1 in _chunks(vq0 * 128, 1024, 512):
                                nc.tensor.matmul(
                                    st[:, hh * 1024 + c0 : hh * 1024 + c1],
                                    lhs,
                                    QT_sb[
                                        hh * 64 : (hh + 1) * 64,
                                        hp,
                                        j * 1024 + c0 : j * 1024 + c1,
                                    ],
                                    start=True,
                                    stop=True,
                                )
                        p_t = sb.tile([128, 2048], BF16, tag="p", bufs=3)
                        st3 = st[:].rearrange("p (h c) -> p h c", h=2)
                        pt3 = p_t[:].rearrange("p (h c) -> p h c", h=2)
                        nc.scalar.activation(
                            pt3[:, :, vq0 * 128 : 1024],
                            st3[:, :, vq0 * 128 : 1024],
                            Exp,
                            scale=float(1.0 / np.sqrt(HD)),
                        )
                        diag = causal and ki >= QG * j
                        if diag:
                            for hh in range(2):
                                dsl = p_t[
                                    :, hh * 1024 + vq0 * 128 : hh * 1024 + vq0 * 128 + 128
                                ]
                                nc.vector.tensor_mul(dsl, dsl, triu_sb[:])
                        for hh in range(2):
                            h = 2 * hp + hh
                            # per-column accumulation bounds: column t done at
                            # ki == qi (causal) or ki == kmax-1 (full)
                            segs = []
                            if causal:
                                if diag:
                                    segs.append((vq0, vq0 + 1, True))
                                    lo = vq0 + 1
                                else:
                                    lo = vq0
                                last = ki == kmax - 1
                                for c0, c1 in _chunks(lo, QG, 4):
                                    segs.append((c0, c1, last))
                            else:
                                for c0, c1 in _chunks(0, QG, 4):
                                    segs.append((c0, c1, ki == kmax - 1))
                            for t0, t1, stop in segs:
                                nc.tensor.matmul(
                                    ctxs[hh][:, t0 * 128 : t1 * 128],
                                    V_sb[:, h, ki, :],
                                    p_t[:, hh * 1024 + t0 * 128 : hh * 1024 + t1 * 128],
                                    start=(ki == 0),
                                    stop=stop,
                                    skip_group_check=True,
                                )
                    # normalization: ctx[:HD] /= ctx[HD] per token column
                    for hh in range(2):
                        h = 2 * hp + hh
                        rz = sb.tile([65, 2, QG * 128], BF16, tag="rz", bufs=2)
                        zps = ps.tile([HD, QG * 128], F32, tag="s", bufs=1, name="zps")
                        if EXACT_Z:
                            rzf = sb.tile([65, QG * 128], F32, tag="rzf", bufs=2)
                            nc.vector.reciprocal(
                                rzf[64:65, :], ctxs[hh][HD : HD + 1, :]
                            )
                            nc.vector.tensor_copy(rz[64:65, 0, :], rzf[64:65, :])
                            nc.vector.tensor_sub(
                                rz[64:65, 1, :], rzf[64:65, :], rz[64:65, 0, :]
                            )
                            parts = (0, 1)
                        else:
                            nc.vector.reciprocal(
                                rz[64:65, 0, :], ctxs[hh][HD : HD + 1, :]
                            )
                            parts = (0,)
                        for c0, c1 in _chunks(0, QG * 128, 512):
                            for pi, part in enumerate(parts):
                                nc.tensor.matmul(
                                    zps[:, c0:c1],
                                    ones_sb[64:65, 0:HD],
                                    rz[64:65, part, c0:c1],
                                    start=(pi == 0),
                                    stop=(pi == len(parts) - 1),
                                )
                        zsb = sb.tile([HD, QG * 128], F32, tag="zsb", bufs=2)
                        nc.vector.tensor_copy(zsb[:], zps[:])
                        nc.vector.tensor_mul(ctxn[h][:], ctxs[hh][0:HD, :], zsb[:])
                # ---- output projection for this pass's q tiles -----------
                for t in range(QG):
                    qi = QG * j + t
                    po = ps.tile([128, D], F32, tag="ctx", bufs=2)
                    for h in range(4):
                        nc.tensor.matmul(
                            po[:],
                            ctxn[h][:, t * 128 : (t + 1) * 128],
                            wo_sb[:, h, :],
                            start=(h == 0),
                            stop=(h == 3),
                        )
                    ob = sb.tile([128, D], F32, tag="osb", bufs=3)
                    nc.vector.tensor_copy(ob[:], po[:])
                    nc.sync.dma_start(out[qi * 128 : (qi + 1) * 128, :], ob[:])

    nc.compile()
    return nc


def get_nc(causal=True):
    if causal not in _CACHE:
        _CACHE[causal] = _build(causal)
    return _CACHE[causal]


def _to_bf16(a):
    return np.asarray(a, np.float32).astype(ml_dtypes.bfloat16)


def _make_in_maps(inputs):
    q = np.asarray(inputs["q"], np.float32)
    k = np.asarray(inputs["k"], np.float32)
    v = np.asarray(inputs["v"], np.float32)
    Wq = np.asarray(inputs["Wq"], np.float32)
    bq = np.asarray(inputs["bq"], np.float32)
    Wk = np.asarray(inputs["Wk"], np.float32)
    bk = np.asarray(inputs["bk"], np.float32)
    Wv = np.asarray(inputs["Wv"], np.float32)
    bv = np.asarray(inputs["bv"], np.float32)
    Wo = np.asarray(inputs["Wo"], np.float32)

    triu_np = np.triu(np.ones((128, 128), np.float32)).astype(ml_dtypes.bfloat16)
    in_maps = []
    for c in range(8):
        b_, g = divmod(c, 2)
        sel = slice(g * DG, (g + 1) * DG)
        bqk_np = np.stack(
            [
                bq[sel][0:128],
                bq[sel][128:256],
                bk[sel][0:128],
                bk[sel][128:256],
            ],
            axis=1,
        ).astype(np.float32)
        wo_g = Wo[:, sel].T.reshape(4, HD, D).transpose(1, 0, 2)
        in_maps.append(
            {
                "qb": _to_bf16(q[b_]),
                "kb": _to_bf16(k[b_]),
                "vb": _to_bf16(v[b_]),
                "wq": _to_bf16(Wq[sel, :].T),
                "wk": _to_bf16(Wk[sel, :].T),
                "wv": _to_bf16(Wv[sel, :].T),
                "bqk": bqk_np,
                "bv": _to_bf16(bv[sel]).reshape(1, DG),
                "wo4": _to_bf16(wo_g),
                "triu": triu_np,
            }
        )
    return in_maps


def kernel(q, k, v, mask, Wq, bq, Wk, bk, Wv, bv, Wo, bo):
    mask = np.asarray(mask)
    bo = np.asarray(bo, np.float32)

    tril = np.tril(np.ones((L, L), mask.dtype))
    causal = all(np.array_equal(mask[b_], tril) for b_ in range(B))
    if not causal:
        assert mask.all(), (
            "only causal (tril) or all-ones masks are supported by this kernel"
        )
    nc = get_nc(causal)

    in_maps = _make_in_maps(
        dict(q=q, k=k, v=v, Wq=Wq, bq=bq, Wk=Wk, bk=bk, Wv=Wv, bv=bv, Wo=Wo)
    )
    res = run_bass_kernel_spmd(nc, in_maps, core_ids=list(range(8)))
    outs = [res.results[c]["out"] for c in range(8)]
    full = np.empty((B, L, D), np.float32)
    for b_ in range(B):
        full[b_] = outs[2 * b_] + outs[2 * b_ + 1] + bo[None, :]
    return full


# revision 17
# speedup vs baseline: 1.6559x; 1.6559x over previous
"""Multi-head attention (B=4, L=2048, D=512, H=8) on 8 Trainium2 NeuronCores.

Sharding: data-parallel over batch (4) x tensor-parallel over heads (2 groups
of 4 heads). Core c handles batch c//2, head-group c%2. Each core computes
   Q^T/K^T (feature-major) and V (token-major) projections for its 4 heads,
   causal attention with scores kept transposed (S^T = K Q^T) so softmax'd
   probabilities feed the PV matmul directly as the moving operand,
   denominators via a ones-column appended to V, and a partial output
   projection over its 256 features.
Host sums the two head-group partials per batch and adds the output bias.

Compute is bf16 (fp32 accumulation in PSUM); softmax runs in fp32 on the
scalar engine with the 1/sqrt(hd) scale fused into the exp activation.
"""
import sys

sys.path.insert(0, "/opt/trn_rl_repo")

import numpy as np
import ml_dtypes

import concourse.bacc as bacc
import concourse.tile as tile
import concourse.mybir as mybir
from concourse import library_config
from concourse.bass_utils import run_bass_kernel_spmd

BF16 = mybir.dt.bfloat16
F32 = mybir.dt.float32
Exp = mybir.ActivationFunctionType.Exp

B, L, D, H = 4, 2048, 512, 8
HD = D // H          # 64
DG = D // 2          # 256 features per head-group
NT = L // 128        # 16 token tiles
QG = 4               # q-tiles per pass
NPASS = NT // QG
SCALE = float(1.0 / np.sqrt(HD))

_CACHE = {}


def _build(causal, dbg=False):
    nc = bacc.Bacc("TRN2", target_bir_lowering=False, debug=False, num_devices=8)
    dt = nc.dram_tensor
    dbg_out = {}
    if dbg:
        dbg_out["KT"] = dt("d_KT", [128, 2, L], BF16, kind="ExternalOutput").ap()
        dbg_out["QT"] = dt("d_QT", [128, 2, L], BF16, kind="ExternalOutput").ap()
        dbg_out["V"] = dt("d_V", [128, 4 * NT * (HD + 1)], BF16, kind="ExternalOutput").ap()
        dbg_out["pt"] = dt("d_pt", [128, 2 * QG * 128], BF16, kind="ExternalOutput").ap()
        dbg_out["ctx"] = dt("d_ctx", [HD + 1, 2 * QG * 128], F32, kind="ExternalOutput").ap()
        dbg_out["rz"] = dt("d_rz", [1, 2 * QG * 128], F32, kind="ExternalOutput").ap()
        dbg_out["zb"] = dt("d_zb", [HD, 2 * QG * 128], F32, kind="ExternalOutput").ap()
        dbg_out["ctxn"] = dt("d_ctxn", [HD, 4 * QG * 128], BF16, kind="ExternalOutput").ap()
    qb = dt("qb", [L, D], BF16, kind="ExternalInput").ap()
    kb = dt("kb", [L, D], BF16, kind="ExternalInput").ap()
    vb = dt("vb", [L, D], BF16, kind="ExternalInput").ap()
    wq = dt("wq", [D, DG], BF16, kind="ExternalInput").ap()
    wk = dt("wk", [D, DG], BF16, kind="ExternalInput").ap()
    wv = dt("wv", [D, DG], BF16, kind="ExternalInput").ap()
    bqk = dt("bqk", [128, 4], F32, kind="ExternalInput").ap()
    bv = dt("bv", [1, DG], BF16, kind="ExternalInput").ap()
    wo4 = dt("wo4", [HD, 4, D], BF16, kind="ExternalInput").ap()
    triu = dt("triu", [128, 128], BF16, kind="ExternalInput").ap()
    out = dt("out", [L, D], F32, kind="ExternalOutput").ap()

    with tile.TileContext(nc) as tc:
        with (
            tc.tile_pool(name="sb", bufs=1) as sb,
            tc.tile_pool(name="ps", space="PSUM", bufs=1) as ps,
        ):
            nc.gpsimd.load_library(library_config.attn)

            # ---- constants / weights -------------------------------------
            wq_sb = sb.tile([128, 4, DG], BF16, tag="wq")
            wk_sb = sb.tile([128, 4, DG], BF16, tag="wk")
            wv_sb = sb.tile([128, 4, DG], BF16, tag="wv")
            nc.sync.dma_start(wq_sb[:], wq.rearrange("(s p) n -> p s n", p=128))
            nc.sync.dma_start(wk_sb[:], wk.rearrange("(s p) n -> p s n", p=128))
            nc.sync.dma_start(wv_sb[:], wv.rearrange("(s p) n -> p s n", p=128))
            bqk_sb = sb.tile([128, 4], F32, tag="bqk")
            nc.sync.dma_start(bqk_sb[:], bqk[:])
            bv_sb = sb.tile([1, DG], BF16, tag="bv")
            nc.sync.dma_start(bv_sb[:], bv[:])
            wo_sb = sb.tile([HD, 4, D], BF16, tag="wo")
            nc.sync.dma_start(wo_sb[:], wo4[:])
            triu_sb = sb.tile([128, 128], BF16, tag="triu")
            nc.sync.dma_start(triu_sb[:], triu[:])
            ones_sb = sb.tile([1, 128], BF16, tag="ones")
            nc.vector.memset(ones_sb[:], 1.0)

            # ---- transposed activation loads (feature-major slabs) -------
            kT, qT, vT = [], [], []
            for s in range(4):
                t = sb.tile([128, L], BF16, tag="xT", bufs=12, name=f"kT{s}")
                nc.sync.dma_start(t[:], kb[:, s * 128 : (s + 1) * 128], transpose=True)
                kT.append(t)
            for s in range(4):
                t = sb.tile([128, L], BF16, tag="xT", bufs=12, name=f"vT{s}")
                nc.sync.dma_start(t[:], vb[:, s * 128 : (s + 1) * 128], transpose=True)
                vT.append(t)
            for s in range(4):
                t = sb.tile([128, L], BF16, tag="xT", bufs=12, name=f"qT{s}")
                nc.sync.dma_start(t[:], qb[:, s * 128 : (s + 1) * 128], transpose=True)
                qT.append(t)

            # ---- K^T / Q^T projections (feature-major outputs) -----------
            KT_sb = sb.tile([128, 2, L], BF16, tag="KT")
            QT_sb = sb.tile([128, 2, L], BF16, tag="QT")
            pi = 0
            for xT, w_sb, bcol, dst in ((kT, wk_sb, 2, KT_sb), (qT, wq_sb, 0, QT_sb)):
                for m in range(2):
                    for n in range(4):
                        pqk = ps.tile(
                            [128, 512], F32, tag=("po", "ctx")[pi % 2], bufs=2,
                            name="pqk",
                        )
                        pi += 1
                        for s in range(4):
                            nc.tensor.matmul(
                                pqk[:],
                                w_sb[:, s, m * 128 : (m + 1) * 128],
                                xT[s][:, n * 512 : (n + 1) * 512],
                                start=(s == 0),
                                stop=(s == 3),
                            )
                        nc.vector.tensor_scalar_add(
                            dst[:, m, n * 512 : (n + 1) * 512],
                            pqk[:],
                            bqk_sb[:, bcol + m : bcol + m + 1],
                        )

            # ---- V projection (token-major, ones column for Z) -----------
            V_sb = sb.tile([128, 4, NT, HD + 1], BF16, tag="V")
            for tt in range(NT):
                pv = ps.tile(
                    [128, DG], F32, tag=("po", "ctx")[pi % 2], bufs=2, name="pv"
                )
                pi += 1
                for s in range(4):
                    nc.tensor.matmul(
                        pv[:],
                        vT[s][:, tt * 128 : (tt + 1) * 128],
                        wv_sb[:, s, :],
                        start=(s == 0),
                        stop=False,
                    )
                nc.tensor.matmul(
                    pv[:], ones_sb[0:1, :], bv_sb[:], start=False, stop=True
                )
                nc.vector.tensor_copy(
                    V_sb[:, :, tt, 0:HD],
                    pv[:].rearrange("p (h d) -> p h d", h=4),
                )
            nc.vector.memset(V_sb[:, :, :, HD : HD + 1], 1.0)

            if dbg:
                nc.sync.dma_start(dbg_out["KT"][:], KT_sb[:])
                nc.sync.dma_start(dbg_out["QT"][:], QT_sb[:])
                nc.sync.dma_start(
                    dbg_out["V"][:], V_sb[:].rearrange("p a b c -> p (a b c)")
                )

            # ---- attention ----------------------------------------------
            for j in range(NPASS):
                ctxn = [
                    sb.tile(
                        [HD, QG * 128], BF16, tag=f"ctxn{h}", bufs=2, name=f"ctxn{h}"
                    )
                    for h in range(4)
                ]
                for hp in range(2):
                    kmax = QG * (j + 1) if causal else NT
                    ctxs = [
                        ps.tile(
                            [HD + 1, QG * 128], F32, tag="ctx", bufs=2, name=f"ctx{i}"
                        )
                        for i in range(2)
                    ]
                    for ki in range(kmax):
                        vq0 = max(ki - QG * j, 0) if causal else 0
                        c0 = vq0 * 128
                        st = ps.tile([128, 2, QG * 128], F32, tag="s", bufs=2, name="st")
                        for hh in range(2):
                            nc.tensor.matmul(
                                st[:, hh, c0:],
                                KT_sb[
                                    hh * 64 : (hh + 1) * 64,
                                    hp,
                                    ki * 128 : (ki + 1) * 128,
                                ],
                                QT_sb[
                                    hh * 64 : (hh + 1) * 64,
                                    hp,
                                    j * QG * 128 + c0 : (j + 1) * QG * 128,
                                ],
                                start=True,
                                stop=True,
                            )
                        p_t = sb.tile([128, 2, QG * 128], BF16, tag="p", bufs=3, name="p_t")
                        nc.scalar.activation(
                            p_t[:, :, c0:], st[:, :, c0:], Exp, scale=SCALE
                        )
                        if dbg and j == 0 and hp == 0 and ki == 0:
                            nc.sync.dma_start(
                                dbg_out["pt"][:], p_t[:].rearrange("p a b -> p (a b)")
                            )
                        diag = causal and ki >= QG * j
                        if diag:
                            for hh in range(2):
                                dsl = p_t[:, hh, c0 : c0 + 128]
                                nc.vector.tensor_mul(dsl, dsl, triu_sb[:])
                        for hh in range(2):
                            h = 2 * hp + hh
                            segs = []
                            if diag:
                                segs.append((vq0, vq0 + 1, True))
                                lo = vq0 + 1
                            else:
                                lo = vq0
                            if lo < QG:
                                segs.append((lo, QG, (not causal) and ki == kmax - 1))
                            for t0, t1, stop in segs:
                                nc.tensor.matmul(
                                    ctxs[hh][:, t0 * 128 : t1 * 128],
                                    V_sb[:, h, ki, :],
                                    p_t[:, hh, t0 * 128 : t1 * 128],
                                    start=(ki == 0),
                                    stop=stop,
                                    skip_group_check=True,
                                )
                    for hh in range(2):
                        h = 2 * hp + hh
                        zrow = sb.tile([1, QG * 128], F32, tag="zrow", bufs=3, name="zrow")
                        nc.vector.tensor_copy(zrow[:], ctxs[hh][HD : HD + 1, :])
                        rzf = sb.tile([1, QG * 128], F32, tag="rz", bufs=3, name="rzf")
                        nc.vector.reciprocal_approx_fast(rzf[:], zrow[:])
                        zb = sb.tile([HD, QG * 128], F32, tag="zb", bufs=3, name="zb")
                        nc.gpsimd.partition_broadcast(zb[:], rzf[:])
                        if dbg and j == 0 and hp == 0:
                            cs = sb.tile(
                                [HD + 1, QG * 128], F32, tag="dbgc", bufs=2, name="cs"
                            )
                            nc.vector.tensor_copy(cs[:], ctxs[hh][:])
                            sl = slice(hh * QG * 128, (hh + 1) * QG * 128)
                            nc.sync.dma_start(dbg_out["ctx"][:, sl], cs[:])
                            nc.sync.dma_start(dbg_out["rz"][:, sl], rzf[:])
                            nc.sync.dma_start(dbg_out["zb"][:, sl], zb[:])
                        nc.vector.tensor_mul(ctxn[h][:], ctxs[hh][0:HD, :], zb[:])
                if dbg and j == 0:
                    for h in range(4):
                        nc.sync.dma_start(
                            dbg_out["ctxn"][:, h * QG * 128 : (h + 1) * QG * 128],
                            ctxn[h][:],
                        )
                # ---- output projection for this pass's q tiles -----------
                for t in range(QG):
                    qi = QG * j + t
                    po = ps.tile([128, D], F32, tag="po", bufs=2, name="po")
                    for h in range(4):
                        nc.tensor.matmul(
                            po[:],
                            ctxn[h][:, t * 128 : (t + 1) * 128],
                            wo_sb[:, h, :],
                            start=(h == 0),
                            stop=(h == 3),
                        )
                    ob = sb.tile([128, D], F32, tag="osb", bufs=3, name="ob")
                    nc.vector.tensor_copy(ob[:], po[:])
                    nc.sync.dma_start(out[qi * 128 : (qi + 1) * 128, :], ob[:])

    nc.compile()
    return nc


def get_nc(causal=True):
    if causal not in _CACHE:
        _CACHE[causal] = _build(causal)
    return _CACHE[causal]


def _to_bf16(a):
    return np.asarray(a, np.float32).astype(ml_dtypes.bfloat16)


def _make_in_maps(inputs):
    q = np.asarray(inputs["q"], np.float32)
    k = np.asarray(inputs["k"], np.float32)
    v = np.asarray(inputs["v"], np.float32)
    Wq = np.asarray(inputs["Wq"], np.float32)
    bq = np.asarray(inputs["bq"], np.float32)
    Wk = np.asarray(inputs["Wk"], np.float32)
    bk = np.asarray(inputs["bk"], np.float32)
    Wv = np.asarray(inputs["Wv"], np.float32)
    bv = np.asarray(inputs["bv"], np.float32)
    Wo = np.asarray(inputs["Wo"], np.float32)

    triu_np = np.triu(np.ones((128, 128), np.float32)).astype(ml_dtypes.bfloat16)
    in_maps = []
    for c in range(8):
        b_, g = divmod(c, 2)
        sel = slice(g * DG, (g + 1) * DG)
        bqk_np = np.stack(
            [
                bq[sel][0:128],
                bq[sel][128:256],
                bk[sel][0:128],
                bk[sel][128:256],
            ],
            axis=1,
        ).astype(np.float32)
        wo_g = Wo[:, sel].T.reshape(4, HD, D).transpose(1, 0, 2)
        in_maps.append(
            {
                "qb": _to_bf16(q[b_]),
                "kb": _to_bf16(k[b_]),
                "vb": _to_bf16(v[b_]),
                "wq": _to_bf16(Wq[sel, :].T),
                "wk": _to_bf16(Wk[sel, :].T),
                "wv": _to_bf16(Wv[sel, :].T),
                "bqk": bqk_np,
                "bv": _to_bf16(bv[sel]).reshape(1, DG),
                "wo4": _to_bf16(wo_g),
                "triu": triu_np,
            }
        )
    return in_maps


def kernel(q, k, v, mask, Wq, bq, Wk, bk, Wv, bv, Wo, bo):
    mask = np.asarray(mask)
    bo = np.asarray(bo, np.float32)

    tril = np.tril(np.ones((L, L), mask.dtype))
    causal = all(np.array_equal(mask[b_], tril) for b_ in range(B))
    if not causal:
        assert mask.all(), (
            "only causal (tril) or all-ones masks are supported by this kernel"
        )
    nc = get_nc(causal)

    in_maps = _make_in_maps(
        dict(q=q, k=k, v=v, Wq=Wq, bq=bq, Wk=Wk, bk=bk, Wv=Wv, bv=bv, Wo=Wo)
    )
    res = run_bass_kernel_spmd(nc, in_maps, core_ids=list(range(8)))
    outs = [res.results[c]["out"] for c in range(8)]
    full = np.empty((B, L, D), np.float32)
    for b_ in range(B):
        full[b_] = outs[2 * b_] + outs[2 * b_ + 1] + bo[None, :]
    return full
